# revision 1
# baseline (speedup 1.0000x reference)
"""Trainium2 Bass kernel for nn_EquivariantMessageLayer (gnn_message_passing).

Strategy (destination-sharded):
  - 8 NeuronCores; core c owns output nodes [6250*c, 6250*(c+1)).
  - Host: route each edge to the core owning its target; sort by target;
    pack into groups whose edges all target a <=128-node window, split into
    two halves by source id (int16 gather addressing); build per-slot side
    arrays (radial^T with a ones-row for the bias, relative targets,
    edge vectors, gather indices, flush row indices).
  - Device phase 0: feat_w = Linear(SiLU-scaled(Linear(nf)))) computed with
    the stationary operand trick (no transposes) into a [50048, 384] bf16
    table whose other half holds vector_features (host-filled).
  - Device phase 1 per group: dma_gather table rows by source; radial
    projection as bf16 matmul with folded scales/bias; per-edge products on
    DVE (batched across the group); segment-sum via one-hot matmul
    accumulated in PSUM; flush [128, 256] f32 rows via indirect scatter to
    host-computed row indices (disjoint across groups; padding rows go to a
    trash area).
  - No collectives: per-core outputs are disjoint row ranges, concatenated
    on the host.
"""
import sys
sys.path.insert(0, '/opt/trn_rl_repo')
import math
import numpy as np
import ml_dtypes

H = 64
R = 32
N = 50000
E = 800000
NC = 8
NPC = N // NC            # 6250 nodes per core
NH = 25024               # half-table rows (int16-addressable)
NPAD = 2 * NH            # 50048 padded node count
INV_SQRT_3 = 1.0 / math.sqrt(3.0)
INV_SQRT_H = 1.0 / math.sqrt(H)
SILU_SCALE = 1.0 / 0.6

CA = 7                   # gather columns for half A (896 slots)
CB = 7
GC = CA + CB             # 14 tile-columns per group (1792 slots)
SLOTS = GC * 128
HALF_SLOTS = CA * 128
DUMMY_TGT = 300.0
OUT_ROWS = 6400
TRASH_BASE = 6272

bf16 = ml_dtypes.bfloat16


def _pack_core(src, tgt_local, radial, ev, ng_fixed=None):
    order = np.argsort(tgt_local, kind='stable')
    src = src[order]; tgt_local = tgt_local[order]
    radial = radial[order]; ev = ev[order]

    node_start = np.searchsorted(tgt_local, np.arange(NPC + 1))
    is_a = src < NH

    groups = []
    b = 0
    while b < NPC:
        nA = nB = 0
        n = b
        while n < NPC and (n - b) < 128:
            e0, e1 = node_start[n], node_start[n + 1]
            a_n = int(is_a[e0:e1].sum()); b_n = (e1 - e0) - a_n
            if nA + a_n > HALF_SLOTS or nB + b_n > HALF_SLOTS:
                break
            nA += a_n; nB += b_n
            n += 1
        assert n > b, "single node exceeds group capacity"
        e0, e1 = node_start[b], node_start[n]
        eids = np.arange(e0, e1)
        groups.append((b, n - b, eids[is_a[e0:e1]], eids[~is_a[e0:e1]]))
        b = n

    ng = len(groups)
    if ng_fixed is None:
        ng_fixed = ng
    assert ng <= ng_fixed

    radial_aug = np.zeros((33, ng_fixed * SLOTS), np.float32)
    tgt_rel = np.full((128, ng_fixed * GC), DUMMY_TGT, np.float32)
    ev_pk = np.zeros((128, ng_fixed * GC * 3), np.float32)
    rowidx = np.zeros((128, ng_fixed), np.int32)
    idx_w = np.zeros((16, ng_fixed * 2 * (HALF_SLOTS // 16)), np.int16)
    ncol = HALF_SLOTS // 16
    jw = np.arange(HALF_SLOTS)

    p = np.arange(128)
    for g in range(ng_fixed):
        if g < ng:
            bg, span, eA, eB = groups[g]
        else:
            bg, span, eA, eB = NPC, 0, np.empty(0, np.int64), np.empty(0, np.int64)
        rowidx[:, g] = np.where(p < span, bg + p, TRASH_BASE + p)
        for half, eids in ((0, eA), (1, eB)):
            ns = eids.shape[0]
            sl = np.arange(ns) + half * HALF_SLOTS
            pp = sl % 128
            cc = sl // 128
            radial_aug[:R, g * SLOTS + sl] = radial[eids].T
            radial_aug[32, g * SLOTS + sl] = 1.0
            tgt_rel[pp, g * GC + cc] = (tgt_local[eids] - bg).astype(np.float32)
            for d in range(3):
                ev_pk[pp, (g * GC + cc) * 3 + d] = ev[eids, d]
            idxs = np.zeros(HALF_SLOTS, np.int16)
            idxs[:ns] = (src[eids] - half * NH).astype(np.int16)
            base = (g * 2 + half) * ncol
            idx_w[jw % 16, base + jw // 16] = idxs

    return dict(ngroups=ng, idx=np.tile(idx_w, (8, 1)),
                radial_aug=radial_aug.astype(bf16), tgt_rel=tgt_rel,
                ev=ev_pk.astype(bf16), rowidx=rowidx)


def _host_prep(node_features, vector_features, edge_index, edge_radial,
               edge_vectors, Wf1, bf1, Wf2, bf2, Wr, br, ng_fixed=None):
    src = np.asarray(edge_index[0]).astype(np.int64)
    tgt = np.asarray(edge_index[1]).astype(np.int64)
    edge_radial = np.asarray(edge_radial, np.float32)
    edge_vectors = np.asarray(edge_vectors, np.float32)
    node_features = np.asarray(node_features, np.float32)
    vector_features = np.asarray(vector_features, np.float32)
    Wf1 = np.asarray(Wf1, np.float32); bf1 = np.asarray(bf1, np.float32)
    Wf2 = np.asarray(Wf2, np.float32); bf2 = np.asarray(bf2, np.float32)
    Wr = np.asarray(Wr, np.float32); br = np.asarray(br, np.float32)

    core_of = tgt // NPC
    packs = []
    for c in range(NC):
        m = core_of == c
        packs.append((src[m], tgt[m] - c * NPC, edge_radial[m], edge_vectors[m]))

    if ng_fixed is None:
        ng_fixed = 0
        probe = [_pack_core(*packs[c]) for c in range(NC)]
        ng_fixed = max(t['ngroups'] for t in probe) + 1
        del probe
    packed = [_pack_core(*packs[c], ng_fixed=ng_fixed) for c in range(NC)]

    nfT_aug = np.zeros((65, NPAD), np.float32)
    nfT_aug[:H, :N] = node_features.T
    nfT_aug[64, :] = 1.0
    nfT_aug = nfT_aug.astype(bf16)

    C_init = np.zeros((NPAD, 384), bf16)
    C_init[:N, 192:384] = vector_features.reshape(N, 192).astype(bf16)

    Wf1_aug = np.zeros((65, 32), np.float32)
    Wf1_aug[:H] = Wf1
    Wf1_aug[64] = bf1
    Wf1_aug = Wf1_aug.astype(bf16)

    Wf2_aug = np.zeros((33, 192), np.float32)
    Wf2_aug[:32] = Wf2 * SILU_SCALE
    Wf2_aug[32] = bf2
    Wf2_aug = Wf2_aug.astype(bf16)

    scales = np.concatenate([
        np.full(64, INV_SQRT_3 * INV_SQRT_H),
        np.full(64, INV_SQRT_3 * INV_SQRT_H),
        np.full(64, INV_SQRT_3),
    ]).astype(np.float32)
    Wr_aug = np.zeros((33, 192), np.float32)
    Wr_aug[:R] = Wr * scales[None, :]
    Wr_aug[32] = br * scales
    Wr_aug = Wr_aug.astype(bf16)

    in_maps = []
    for c in range(NC):
        pk = packed[c]
        in_maps.append({
            "nfT_aug": nfT_aug, "C": C_init, "Wf1_aug": Wf1_aug,
            "Wf2_aug": Wf2_aug, "Wr_aug": Wr_aug, "idx": pk['idx'],
            "radial_aug": pk['radial_aug'], "tgt_rel": pk['tgt_rel'],
            "ev": pk['ev'], "rowidx": pk['rowidx'],
        })
    return in_maps, ng_fixed


def _build_kernel(ng, reps=1):
    import concourse.bass as bass
    import concourse.bacc as bacc
    import concourse.mybir as mybir
    import concourse.tile as tile

    f32 = mybir.dt.float32
    b16 = mybir.dt.bfloat16
    i16 = mybir.dt.int16
    i32 = mybir.dt.int32
    Alu = mybir.AluOpType
    Act = mybir.ActivationFunctionType

    nc = bacc.Bacc("TRN2", target_bir_lowering=False, debug=False, num_devices=NC)

    nfT_aug = nc.dram_tensor("nfT_aug", [65, NPAD], b16, kind="ExternalInput").ap()
    C = nc.dram_tensor("C", [NPAD, 384], b16, kind="ExternalInput").ap()
    Wf1_aug = nc.dram_tensor("Wf1_aug", [65, 32], b16, kind="ExternalInput").ap()
    Wf2_aug = nc.dram_tensor("Wf2_aug", [33, 192], b16, kind="ExternalInput").ap()
    Wr_aug = nc.dram_tensor("Wr_aug", [33, 192], b16, kind="ExternalInput").ap()
    idx = nc.dram_tensor("idx", [128, ng * 2 * (HALF_SLOTS // 16)], i16,
                         kind="ExternalInput").ap()
    radial_aug = nc.dram_tensor("radial_aug", [33, ng * SLOTS], b16,
                                kind="ExternalInput").ap()
    tgt_rel = nc.dram_tensor("tgt_rel", [128, ng * GC], f32, kind="ExternalInput").ap()
    ev = nc.dram_tensor("ev", [128, ng * GC * 3], b16, kind="ExternalInput").ap()
    rowidx = nc.dram_tensor("rowidx", [128, ng], i32, kind="ExternalInput").ap()
    out = nc.dram_tensor("out", [OUT_ROWS, 256], f32, kind="ExternalOutput").ap()

    NT0 = NPAD // 128

    with tile.TileContext(nc) as tc:
        with (
            tc.tile_pool(name="res", bufs=1) as res,
            tc.tile_pool(name="p0", bufs=3) as p0,
            tc.tile_pool(name="htp", bufs=4) as htp,
            tc.tile_pool(name="gat", bufs=2) as gat,
            tc.tile_pool(name="wk", bufs=3) as wk,
            tc.tile_pool(name="fl", bufs=2) as fl,
            tc.tile_pool(name="ps", bufs=2, space="PSUM") as ps,
            tc.tile_pool(name="psg", bufs=2, space="PSUM") as psg,
        ):
            wf1_t = res.tile([65, 32], b16)
            nc.sync.dma_start(out=wf1_t[:], in_=Wf1_aug)
            wf2_t = res.tile([33, 192], b16)
            nc.sync.dma_start(out=wf2_t[:], in_=Wf2_aug)
            wr_t = res.tile([33, 192], b16)
            nc.sync.dma_start(out=wr_t[:], in_=Wr_aug)
            idx_t = res.tile([128, ng * 2 * (HALF_SLOTS // 16)], i16)
            nc.sync.dma_start(out=idx_t[:], in_=idx)
            tgt_t = res.tile([128, ng * GC], f32)
            nc.sync.dma_start(out=tgt_t[:], in_=tgt_rel)
            ev_t = res.tile([128, ng * GC * 3], b16)
            nc.sync.dma_start(out=ev_t[:], in_=ev)
            rowidx_t = res.tile([128, ng], i32)
            nc.sync.dma_start(out=rowidx_t[:], in_=rowidx)
            iota_t = res.tile([128, 128], b16)
            nc.gpsimd.iota(iota_t[:], pattern=[[1, 128]], base=0,
                           channel_multiplier=0,
                           allow_small_or_imprecise_dtypes=True)

            zero_t = res.tile([128, 256], f32)
            nc.vector.memset(zero_t[:], 0.0)
            for i in range(0, TRASH_BASE, 128):
                nc.sync.dma_start(out=out[i:i + 128, :], in_=zero_t[:])

            # phase 0
            for t in range(NT0):
                nf_sl = p0.tile([65, 128], b16)
                nc.sync.dma_start(out=nf_sl[:], in_=nfT_aug[:, t * 128:(t + 1) * 128])
                hT_ps = ps.tile([32, 128], f32)
                nc.tensor.matmul(out=hT_ps[:], lhsT=wf1_t[:], rhs=nf_sl[:],
                                 start=True, stop=True)
                hT = htp.tile([33, 128], b16)
                nc.vector.memset(hT[32:33, :], 1.0)
                nc.scalar.activation(out=hT[:32, :], in_=hT_ps[:], func=Act.Silu)
                f_ps = ps.tile([128, 192], f32)
                nc.tensor.matmul(out=f_ps[:], lhsT=hT[:], rhs=wf2_t[:],
                                 start=True, stop=True)
                f_sb = p0.tile([128, 192], b16)
                nc.scalar.copy(out=f_sb[:], in_=f_ps[:])
                nc.sync.dma_start(out=C[t * 128:(t + 1) * 128, 0:192], in_=f_sb[:])

            # phase 1
            ncol = HALF_SLOTS // 16
            for rep in range(reps):
                for g in range(ng):
                    rT = gat.tile([33, SLOTS], b16)
                    nc.sync.dma_start(
                        out=rT[:], in_=radial_aug[:, g * SLOTS:(g + 1) * SLOTS])
                    gt = gat.tile([128, GC, 384], b16)
                    ib = g * 2 * ncol
                    nc.gpsimd.dma_gather(
                        out_ap=gt[:, 0:CA, :], in_ap=C[0:NH, :],
                        idxs_ap=idx_t[:, ib:ib + ncol],
                        num_idxs=HALF_SLOTS, num_idxs_reg=HALF_SLOTS,
                        elem_size=384)
                    nc.gpsimd.dma_gather(
                        out_ap=gt[:, CA:GC, :], in_ap=C[NH:NPAD, :],
                        idxs_ap=idx_t[:, ib + ncol:ib + 2 * ncol],
                        num_idxs=HALF_SLOTS, num_idxs_reg=HALF_SLOTS,
                        elem_size=384)

                    acc = psg.tile([128, 448], f32)
                    rw_all = wk.tile([128, GC, 192], b16)
                    msg_all = wk.tile([128, GC, 448], b16)
                    w12_all = wk.tile([128, GC, 128], b16)
                    for c in range(GC):
                        rw_ps = ps.tile([128, 192], f32)
                        nc.tensor.matmul(
                            out=rw_ps[:], lhsT=rT[:, c * 128:(c + 1) * 128],
                            rhs=wr_t[:], start=True, stop=True)
                        nc.scalar.copy(out=rw_all[:, c, :], in_=rw_ps[:])
                    nc.vector.tensor_tensor(
                        out=w12_all[:], in0=rw_all[:, :, 0:128],
                        in1=gt[:, :, 0:128], op=Alu.mult)
                    nc.vector.tensor_tensor(
                        out=msg_all[:, :, 192:256], in0=rw_all[:, :, 128:192],
                        in1=gt[:, :, 128:192], op=Alu.mult)
                    nc.vector.tensor_tensor(
                        out=msg_all[:, :, 0:192],
                        in0=w12_all[:, :, 0:64].unsqueeze(2)
                            .broadcast_to([128, GC, 3, 64]),
                        in1=gt[:, :, 192:384].rearrange("p c (d h) -> p c d h", d=3),
                        op=Alu.mult)
                    nc.vector.tensor_tensor(
                        out=msg_all[:, :, 256:448],
                        in0=w12_all[:, :, 64:128].unsqueeze(2)
                            .broadcast_to([128, GC, 3, 64]),
                        in1=ev_t[:, (g * GC) * 3:(g + 1) * GC * 3]
                            .rearrange("p (c d) -> p c d", d=3).unsqueeze(3)
                            .broadcast_to([128, GC, 3, 64]),
                        op=Alu.mult)
                    for c in range(GC):
                        S = wk.tile([128, 128], b16)
                        nc.vector.tensor_scalar(
                            out=S[:], in0=iota_t[:],
                            scalar1=tgt_t[:, g * GC + c:g * GC + c + 1],
                            scalar2=None, op0=Alu.is_equal)
                        nc.tensor.matmul(
                            out=acc[:], lhsT=S[:], rhs=msg_all[:, c, :],
                            start=(c == 0), stop=(c == GC - 1))
                    ft = fl.tile([128, 256], f32)
                    nc.scalar.copy(out=ft[:, 0:192], in_=acc[:, 0:192])
                    nc.vector.tensor_tensor(
                        out=ft[:, 0:192], in0=ft[:, 0:192], in1=acc[:, 256:448],
                        op=Alu.add)
                    nc.scalar.copy(out=ft[:, 192:256], in_=acc[:, 192:256])
                    nc.gpsimd.indirect_dma_start(
                        out=out, out_offset=bass.IndirectOffsetOnAxis(
                            ap=rowidx_t[:, g:g + 1], axis=0),
                        in_=ft[:], in_offset=None)
    nc.compile()
    return nc


_CACHE = {}


def kernel(**inputs):
    from concourse.bass_utils import run_bass_kernel_spmd
    in_maps, ng = _host_prep(**inputs)
    key = ng
    if key not in _CACHE:
        _CACHE[key] = _build_kernel(ng)
    nc = _CACHE[key]
    res = run_bass_kernel_spmd(nc, in_maps, list(range(NC)))
    outs = [res.results[c]["out"] for c in range(NC)]
    agg_s = np.concatenate([o[:NPC, 192:256] for o in outs], axis=0)
    agg_v = np.concatenate([o[:NPC, 0:192].reshape(NPC, 3, 64) for o in outs],
                           axis=0)
    return agg_s.astype(np.float32), agg_v.astype(np.float32)
